# revision 68
# baseline (speedup 1.0000x reference)
"""Fused linear + cross-entropy loss (BaseChunkLoss) on 8 trn2 NeuronCores.

Strategy (per the sharding hint: tensor-parallel over vocab):
  - head_weight is sharded 8 ways over the vocab dim: each core handles the
    FULL 8192 tokens x a 4000-entry vocab slice and produces the partial
    sum_{v in shard} exp(logit[t, v]) for every token.  The cross-device
    logsumexp reduction (sum of the 8 partials, then log) plus the weighted
    mean happen on host, standing in for the wrapper's all_reduce.
  - This puts each core's HBM traffic at ~117 MB (full hidden 67 MB + W
    slice 33 MB + target-row gather 17 MB) -- under the fp8 PE roofline of
    ~427 us -- instead of the ~290 MB/core a token-sharded design pays to
    stream the whole 262 MB weight through every core (DMA-bound ~980 us).
  - The W slice is cast to fp8 (x64, e4m3 range) once and stays resident in
    SBUF; hidden^T streams through in 1024-token chunks, cast on the fly.

Device kernel layout: tokens on PSUM partitions, vocab on the free dim.
  stationary lhsT = hidden^T tile [128 d x 2 x 128 tok]   (fp8, DoubleRow)
  moving rhs      = weight^T tile [128 d x 2 x 500 vocab]
  psum [128 tok x 500 vocab] fp32, accumulated over D=2048 in 8 matmuls.
Per 1000-wide vocab group (2 psum banks, 4 groups in flight): DVE does
(psum/64 + bias) in place, ACT computes exp with a fused free-dim row-sum
accumulator into s_cols.  The target logit is computed exactly in f32 as a
DVE rowdot of the core's 1024-token hidden slice against the host-gathered
W[labels] rows; host adds bias[labels].

Schedule notes (tuned against the TimelineSim cost model, HW-verified):
  - Deep stage pools (bufs=4) decouple the serial DMA stream from the fp8
    casts; shallow stages serialize DMA behind cast semaphores (-42 us).
  - Prologue DMA order and compute traversal (exhaustively searched over
    all order-preserving merges of the W-group and h-half streams, with
    the traversal derived from modeled arrival order) keep the in-order
    PE from waiting on far-future transfers.
  - W tile inner stride padded to 4096 (512B-aligned j-stride for DoubleRow
    weight loads).
  - DVE tensor_tensor_reduce is avoided (walrus codegen fails at runtime);
    the rowdot uses tensor_mul + tensor_reduce.

Host-side input prep is layout-only (transpose/slice/gather of rows); all
FLOPs over hidden/weights happen on device inside the measured kernel.

Modeled HW exec time: 494591 ns vs 982556 ns for the token-sharded
baseline (1.99x); relative loss error ~7.6e-5 on hardware.  Remaining idle
(~63 us prologue trickle) is at the structural floor: serial 360 GB/s DMA +
in-order PE + 8-bank PSUM cap consumption at ~8 matmuls per arriving
kp-piece, and column-slab delivery that would fix it cannot fit its stage
buffers in SBUF without giving back more than it gains.
"""
import numpy as np
from contextlib import ExitStack

from concourse import bacc, mybir, tile
from concourse.bass_utils import run_bass_kernel_spmd

F32 = mybir.dt.float32
FP8 = mybir.dt.float8e4
Alu = mybir.AluOpType
Act = mybir.ActivationFunctionType

N_CORES = 8
N_TOK = 8192
D = 2048
V = 32000
P = 128

VSH = V // N_CORES      # 4000 vocab entries per core
TC = N_TOK // N_CORES   # 1024 tokens per core (for the exact tgt rowdot)
KP2 = D // (2 * P)      # 8 DoubleRow contraction steps of K=256
BANK = 500              # vocab columns per psum bank (<= 512 fp32)
BPG = 2                 # banks per vocab group
GV = BPG * BANK         # 1000 vocab per group
NG = VSH // GV          # 4 groups
CHT = 1024              # tokens per streamed hidden chunk
NCH = N_TOK // CHT      # 8 chunks
MBC = CHT // P          # 8 token blocks per chunk
MBT = N_TOK // P        # 64 token blocks total
HSP = 512               # tokens per hidden DMA piece
DHALF = D // 2          # rowdot split for SBUF economy

W_SCALE = 64.0          # fp8 weight pre-scale (e4m3 range)
VPAD = 4096             # W tile inner stride (j-stride must be 512B-aligned)

_DBG_LABELS = {}

# prologue schedule: DMA-stream merge of W groups (W0..W3, bias attached)
# and h chunk halves (Hcs), plus the matching compute traversal
# (chunk, group, half). Overridable for search (prologue_search.py).
_PROLOGUE_ORDER = ("H00", "W0", "H01", "W1", "H10", "H11", "H20", "H21",
                   "W2", "W3")
_PROLOGUE_TRAV = [
    (0, 0, 0), (0, 0, 1), (0, 1, 0), (0, 1, 1),
    (1, 0, 0), (1, 1, 0), (1, 0, 1), (1, 1, 1),
    (2, 0, 0), (2, 1, 0), (2, 0, 1), (2, 1, 1),
    (0, 2, 0), (0, 2, 1), (1, 2, 0), (1, 2, 1), (2, 2, 0), (2, 2, 1),
    (0, 3, 0), (0, 3, 1), (1, 3, 0), (1, 3, 1), (2, 3, 0), (2, 3, 1),
]


def _lab(inst, label):
    try:
        _DBG_LABELS[inst.name] = label
    except Exception:
        pass
    return inst


def _build():
    nc = bacc.Bacc("TRN2", target_bir_lowering=False, debug=False)
    h_d = nc.declare_dram_parameter("h", [D, N_TOK], F32, isOutput=False)
    W_d = nc.declare_dram_parameter("W", [D, VSH], F32, isOutput=False)
    bias_d = nc.declare_dram_parameter("bias", [VSH], F32, isOutput=False)
    hn_d = nc.declare_dram_parameter("hn", [TC, D], F32, isOutput=False)
    wg_d = nc.declare_dram_parameter("wg", [TC, D], F32, isOutput=False)
    s_out = nc.declare_dram_parameter("s_out", [P, MBT * NG + 1], F32,
                                      isOutput=True)
    t_out = nc.declare_dram_parameter("t_out", [P, TC // P * 2], F32, isOutput=True)

    h_r2 = h_d[:].rearrange("(kp j ki) t -> kp ki j t", ki=P, j=2)
    W_r2 = W_d[:].rearrange("(kp j ki) v -> kp ki j v", ki=P, j=2)

    with tile.TileContext(nc) as tc, ExitStack() as ctx:
        wpool = ctx.enter_context(tc.tile_pool(name="w", bufs=1))
        wstage = ctx.enter_context(tc.tile_pool(name="wstage", bufs=4))
        hpool = ctx.enter_context(tc.tile_pool(name="hT", bufs=3))
        hstage = ctx.enter_context(tc.tile_pool(name="hstage", bufs=4))
        bpool = ctx.enter_context(tc.tile_pool(name="bias", bufs=1))
        gpool = ctx.enter_context(tc.tile_pool(name="gath", bufs=2))
        djunk = ctx.enter_context(tc.tile_pool(name="djunk", bufs=1))
        ejunk = ctx.enter_context(tc.tile_pool(name="ejunk", bufs=2))
        pspool = ctx.enter_context(tc.tile_pool(name="ps", bufs=4, space="PSUM"))
        acc = ctx.enter_context(tc.tile_pool(name="acc", bufs=1))

        s_cols = acc.tile([P, MBT * NG + 1], F32, tag="scols")
        t_cols = acc.tile([P, TC // P * 2], F32, tag="tcols")

        bb = bpool.tile([P, VSH], F32, tag="bias")

        def stage_bias(g):
            v0 = g * GV
            nc.sync.dma_start(
                bb[:, v0:v0 + GV], bias_d[v0:v0 + GV].partition_broadcast(P))

        h_tiles = [None] * NCH

        def stage_h_half(c, s):
            # piece order s-outer/kp-inner so early token blocks complete
            # (and unblock their matmuls) before the whole chunk lands
            hc = h_tiles[c]
            for kp in range(KP2):
                t0 = c * CHT + s * HSP
                st = hstage.tile([P, 2, HSP], F32, tag="hstage")
                _lab(nc.sync.dma_start(st[:], h_r2[kp][:, :, t0:t0 + HSP]),
                     f"dma_h c{c} s{s} kp{kp}")
                eng = nc.gpsimd if kp % 2 == 0 else nc.vector
                _lab(eng.tensor_copy(
                    hc[:, kp, :, s * HSP:(s + 1) * HSP], st[:]),
                     f"cast_h c{c} s{s} kp{kp}")

        def stage_h(c):
            hc = hpool.tile([P, KP2, 2, CHT], FP8, tag="hT")
            h_tiles[c] = hc
            for s in range(CHT // HSP):
                stage_h_half(c, s)

        wv = wpool.tile([P, KP2, 2, VPAD], FP8, tag="w")

        def stage_w(g):
            v0 = g * GV
            for kp in range(KP2):
                ws = wstage.tile([P, 2, GV], F32, tag="wstage")
                _lab(nc.sync.dma_start(ws[:], W_r2[kp][:, :, v0:v0 + GV]),
                     f"dma_w g{g} kp{kp}")
                # alternate cast engine per piece (baseline pattern): halves
                # the cast-chain latency behind each W group's arrival
                eng = nc.gpsimd if kp % 2 == 0 else nc.vector
                _lab(eng.tensor_scalar_mul(
                    wv[:, kp, :, v0:v0 + GV], ws[:], W_SCALE),
                     f"cast_w g{g} kp{kp}")

        def compute(c, mm, g):
            m = c * MBC + mm
            pt = pspool.tile([P, BPG, 512], F32, tag="ps")
            lhsT = h_tiles[c][:, :, :, mm * P:(mm + 1) * P]
            for kp in range(KP2):
                for bk in range(BPG):
                    _lab(nc.tensor.matmul(
                        pt[:, bk, 0:BANK], lhsT[:, kp],
                        wv[:, kp, :, g * GV + bk * BANK:g * GV + (bk + 1) * BANK],
                        start=(kp == 0), stop=(kp == KP2 - 1),
                        perf_mode=mybir.MatmulPerfMode.DoubleRow,
                    ), f"mm c{c} m{mm} g{g} kp{kp} bk{bk}")
            psl = pt[:, 0:BPG, 0:BANK]
            bbv = bb[:, g * GV:(g + 1) * GV].rearrange("p (b c) -> p b c", c=BANK)
            _lab(nc.vector.scalar_tensor_tensor(
                psl, psl, 1.0 / W_SCALE, bbv, op0=Alu.mult, op1=Alu.add),
                 f"bias c{c} m{mm} g{g}")
            et = ejunk.tile([P, BPG, BANK], F32, tag="ejunk")
            col = m * NG + g
            _lab(nc.scalar.activation(
                et[:], psl, Act.Exp, accum_out=s_cols[:, col:col + 1]),
                 f"exp c{c} m{mm} g{g}")

        def compute_1bank(c, mm, v0, col):
            pt = pspool.tile([P, BPG, 512], F32, tag="ps")
            lhsT = h_tiles[c][:, :, :, mm * P:(mm + 1) * P]
            for kp in range(KP2):
                _lab(nc.tensor.matmul(
                    pt[:, 0, 0:BANK], lhsT[:, kp],
                    wv[:, kp, :, v0:v0 + BANK],
                    start=(kp == 0), stop=(kp == KP2 - 1),
                    perf_mode=mybir.MatmulPerfMode.DoubleRow,
                ), f"mm1b c{c} m{mm} v{v0} kp{kp}")
            psl = pt[:, 0:1, 0:BANK]
            bbv = bb[:, v0:v0 + BANK].rearrange("p (b c) -> p b c", c=BANK)
            _lab(nc.vector.scalar_tensor_tensor(
                psl, psl, 1.0 / W_SCALE, bbv, op0=Alu.mult, op1=Alu.add),
                 f"bias1b c{c} m{mm} v{v0}")
            et = ejunk.tile([P, BPG, BANK], F32, tag="ejunk")
            _lab(nc.scalar.activation(
                et[:, 0:1, :], psl, Act.Exp, accum_out=s_cols[:, col:col + 1]),
                 f"exp1b c{c} m{mm} v{v0}")

        def rowdot(r):
            # exact f32 target logit for token block r of this core's slice
            # (tensor_mul + tensor_reduce: DVE tensor_tensor_reduce fails in
            # walrus codegen at runtime)
            for hh in range(2):
                hg = gpool.tile([P, DHALF], F32, tag="hg")
                nc.sync.dma_start(
                    hg[:], hn_d[r * P:(r + 1) * P, hh * DHALF:(hh + 1) * DHALF])
                wgt = gpool.tile([P, DHALF], F32, tag="wgt")
                nc.sync.dma_start(
                    wgt[:], wg_d[r * P:(r + 1) * P, hh * DHALF:(hh + 1) * DHALF])
                dj = djunk.tile([P, DHALF], F32, tag="djunk")
                nc.vector.tensor_mul(dj[:], hg[:], wgt[:])
                nc.vector.tensor_reduce(
                    t_cols[:, r * 2 + hh:r * 2 + hh + 1], dj[:],
                    axis=mybir.AxisListType.X, op=Alu.add)

        # -- prologue: interleave W groups, bias slices and h chunks on the
        # DMA queue; traverse compute in the same order the data arrives so
        # the in-order PE stream never waits on a far-future transfer --
        hc = hpool.tile([P, KP2, 2, CHT], FP8, tag="hT")
        h_tiles[0] = hc
        stage_h_half(0, 0)
        stage_w(0)
        stage_bias(0)
        stage_h_half(0, 1)
        stage_w(1)
        stage_bias(1)
        stage_h(1)
        stage_w(2)
        stage_bias(2)
        stage_w(3)
        stage_bias(3)
        stage_h(2)

        for c, g in (
            (0, 0), (0, 1), (1, 0), (1, 1), (0, 2), (1, 2),
            (0, 3), (1, 3), (2, 0), (2, 1), (2, 2), (2, 3),
        ):
            for mm in range(MBC):
                compute(c, mm, g)

        # steady state: prefetch chunk c+1, compute chunk c
        stage_h(3)
        for c in range(3, NCH):
            if c + 1 < NCH:
                stage_h(c + 1)
            for mm in range(MBC):
                for g in range(NG):
                    if c == NCH - 1 and mm == MBC - 1 and g == NG - 1:
                        # final tile: two 1-bank halves so the drain chain
                        # (bias->exp->dma) is half-width after the last matmul
                        compute_1bank(c, mm, g * GV, (c * MBC + mm) * NG + g)
                        compute_1bank(c, mm, g * GV + BANK, MBT * NG)
                    else:
                        compute(c, mm, g)
            # spread the 8 exact-tgt rowdots over mid-stream chunks
            if 3 <= c <= 6:
                rowdot(2 * (c - 3))
                rowdot(2 * (c - 3) + 1)
        nc.sync.dma_start(s_out[:], s_cols[:])
        nc.sync.dma_start(t_out[:], t_cols[:])

    nc.compile()
    return nc


_NC_CACHE = {}


def _get_program():
    if "v" not in _NC_CACHE:
        _NC_CACHE["v"] = _build()
    return _NC_CACHE["v"]


def kernel(hidden_states, head_weight, head_bias, loss_weight, labels,
           chunk_size=None, **_unused):
    hidden = np.asarray(hidden_states, dtype=np.float32)
    W = np.asarray(head_weight, dtype=np.float32)
    bias = np.asarray(head_bias, dtype=np.float32)
    lw = np.asarray(loss_weight, dtype=np.float32)
    labels = np.asarray(labels).astype(np.int64)

    assert hidden.shape == (N_TOK, D) and W.shape == (V, D)

    nc = _get_program()
    Wt = np.ascontiguousarray(W.T)                 # [D, V]
    ht = np.ascontiguousarray(hidden.T)            # [D, N]
    Wg = W[labels]                                 # gathered rows [N, D]
    in_maps = []
    for c in range(N_CORES):
        vsl = slice(c * VSH, (c + 1) * VSH)
        tsl = slice(c * TC, (c + 1) * TC)
        in_maps.append(dict(
            h=ht,
            W=np.ascontiguousarray(Wt[:, vsl]),
            bias=np.ascontiguousarray(bias[vsl]),
            hn=np.ascontiguousarray(hidden[tsl]),
            wg=np.ascontiguousarray(Wg[tsl]),
        ))
    res = run_bass_kernel_spmd(nc, in_maps, list(range(N_CORES)))

    # unshard + host-side scalar combine (the "all_reduce" of the hint):
    # sum the 8 per-core vocab-shard partials of sum_v exp(logit) per token
    s = np.zeros(N_TOK, dtype=np.float64)
    for r in res.results:
        so = r["s_out"].astype(np.float64)
        sc = so[:, :MBT * NG].reshape(P, MBT, NG).sum(axis=2)
        sc[:, MBT - 1] += so[:, MBT * NG]
        s += sc.T.reshape(N_TOK)
    # exact f32 target dot h . W[label] (+ bias) per token
    tgt = np.concatenate([
        r["t_out"].astype(np.float64).reshape(P, TC // P, 2).sum(axis=2)
        .T.reshape(TC)
        for r in res.results])
    tgt = tgt + bias[labels].astype(np.float64)
    lse = np.log(s)
    nll = lse - tgt
    w64 = lw.astype(np.float64)
    loss = (w64 * nll).sum() / max(w64.sum(), 1.0)
    return np.float32(loss)
